# revision 24
# baseline (speedup 1.0000x reference)
"""Embedding-lookup MF model kernel for Trainium2 (8 NeuronCores).

reference math (B = 16384, D = 64):
    u   = user_table[x[:, 0]]          # [B, D]
    v   = item_table[x[:, 1]]          # [B, D]
    out = sigmoid(sum(u * v, -1))      # [B]

Strategy: data-parallel across the batch, with HOST-side index prep that
lets each core fetch its 4096 rows with 8 InstDMAGatherAnt instructions
spread over the 4 SWDGE queues. Q7 descriptor generation costs ~9 ns per
256 B row-descriptor and queue r dispatches to its own Q7 core pair, so 4
queues generate concurrently (~1024 descriptors each, ~4.7 us) — vs ~36 us
for the 32 serialized per-partition indirect DMAs of the naive kernel.

dma_gather takes int16 indices (< 32768) into a row window whose base is a
compile-time AP offset, and writes gathered row i of the instruction to
dst[i % 128, i // 128, :]. To make every index fit in 16 bits:

  - batch rows are sorted globally by user id and dealt to the 8 cores in
    2048-row quantile spans: each core's user ids then span ~12.5k values
    (< 32768 with huge margin), so all u-gathers read one per-core user
    window (sliced host-side and uploaded per core).
  - within a core, its 2048 rows are sorted by item id; the 4 runs of 512
    consecutive sorted item ids each span ~25k values (< 32768), giving
    one v-gather per run from a per-run item window.

Queue r handles positions [512r, 512r+512): one u-gather + one v-gather.

Per-core uploaded table: [5 * 32768, 64] f32 = 40 MB (u window + 4 v
windows). Index tile: [128, 256] int16 (idx i of an instruction lives at
partition i%16, col i//16, replicated 8x down the partition dim for the 8
Q7 cores). Host un-permutes the [128, 16] result tiles at the end.

The mlp Q7 library (which provides dma_gather) is loaded up front; its
~6.5 us IRAM reload overlaps the idx load. Each run's DVE mul + reduce,
ACT sigmoid, and output store pipeline behind the gathers, so only the
last run's chain sits on the tail.
"""

import os

# A previously crashed process can leave the NeuronCores wedged
# (NRT_EXEC_UNIT_UNRECOVERABLE on the next run); requesting a core reset at
# runtime init is harmless otherwise and self-heals that state.
os.environ.setdefault("NEURON_RT_RESET_CORES", "1")

import numpy as np

import concourse.mybir as mybir
import concourse.tile as tile
from concourse import bacc, library_config
from concourse.bass_utils import run_bass_kernel_spmd

N_CORES = 8
P = 128
D = 64
B = 16384
BPC = B // N_CORES  # 2048 batch rows per core
NBLK = BPC // P  # 16 column blocks of 128 batch rows
WIN = 32768  # dma_gather int16 index window (rows)
VRUNS = 4
VRUN = BPC // VRUNS  # 512 positions per v-run
VBLK = VRUN // P  # 4 blocks per v-run
UCOLS = BPC // 16  # 128 idx columns for the u gather
VCOLS = VRUN // 16  # 32 idx columns per v run

_programs: dict = {}


def _build():
    """Single-core program, run SPMD on 8 cores."""
    nc = bacc.Bacc(
        "TRN2",
        target_bir_lowering=False,
        debug=False,
        detect_race_conditions=False,
        num_swdge_queues=4,
    )
    idx = nc.dram_tensor(
        "idx", [P, UCOLS + VRUNS * VCOLS], mybir.dt.int16, kind="ExternalInput"
    )
    tbl = nc.dram_tensor(
        "tbl", [(1 + VRUNS) * WIN, D], mybir.dt.float32, kind="ExternalInput"
    )
    out = nc.dram_tensor("out", [P, NBLK], mybir.dt.float32, kind="ExternalOutput")

    with tile.TileContext(nc) as tc:
        with (
            tc.tile_pool(name="io", bufs=1) as io_pool,
            tc.tile_pool(name="prod", bufs=2) as prod_pool,
        ):
            # Q7 IRAM reload for the gather library; blocks the Q7 cluster
            # ~6.5 us, overlapping the idx transfer below
            nc.gpsimd.load_library(library_config.mlp)
            t_idx = io_pool.tile([P, UCOLS + VRUNS * VCOLS], mybir.dt.int16)
            nc.scalar.dma_start(out=t_idx[:], in_=idx[:])
            tu = io_pool.tile([P, BPC // P * D], mybir.dt.float32)
            tv = io_pool.tile([P, BPC // P * D], mybir.dt.float32)
            t_res = io_pool.tile([P, NBLK], mybir.dt.float32)
            t_bias = io_pool.tile([P, 1], mybir.dt.float32)
            nc.vector.memset(t_bias[:], 0.0)

            tu3 = tu[:].rearrange("p (n d) -> p n d", d=D)
            tv3 = tv[:].rearrange("p (n d) -> p n d", d=D)

            # Q7 descriptor generation runs at ~9 ns/descriptor per core
            # pair, and SWDGE queue r dispatches to its own core pair — so
            # split the work into a (u, v) gather pair per queue: 4 pairs
            # generate concurrently, ~1024 descriptors each. Queue 0's
            # pushes block the Pool sequencer until its pair consumes them,
            # so issue queue 0 last — queues 1-3 then start without the
            # ~4.6 us stagger.
            for r in [1, 2, 3, 0]:
                nc.gpsimd.dma_gather(
                    tu3[:, r * VBLK : (r + 1) * VBLK, :],
                    tbl[0:WIN, :],
                    t_idx[:, r * 2 * VCOLS : r * 2 * VCOLS + VCOLS],
                    VRUN,
                    VRUN,
                    D,
                    queue_num=r,
                )
            for r in [1, 2, 3, 0]:
                c0 = r * 2 * VCOLS + VCOLS
                nc.gpsimd.dma_gather(
                    tv3[:, r * VBLK : (r + 1) * VBLK, :],
                    tbl[(1 + r) * WIN : (2 + r) * WIN, :],
                    t_idx[:, c0 : c0 + VCOLS],
                    VRUN,
                    VRUN,
                    D,
                    queue_num=r,
                )
                w = prod_pool.tile([P, VBLK * D], mybir.dt.float32, tag="w")
                nc.vector.tensor_mul(
                    out=w[:],
                    in0=tu[:, r * VBLK * D : (r + 1) * VBLK * D],
                    in1=tv[:, r * VBLK * D : (r + 1) * VBLK * D],
                )
                rs = t_res[:, r * VBLK : (r + 1) * VBLK]
                nc.vector.reduce_sum(
                    out=rs,
                    in_=w[:].rearrange("p (n d) -> p n d", d=D),
                    axis=mybir.AxisListType.X,
                )
                nc.scalar.activation(
                    out=rs,
                    in_=rs,
                    func=mybir.ActivationFunctionType.Sigmoid,
                    bias=t_bias[:],
                )
                nc.sync.dma_start(
                    out=out[:, r * VBLK : (r + 1) * VBLK], in_=rs
                )

    nc.compile()
    return nc


def _get_program():
    if "p" not in _programs:
        _programs["p"] = _build()
    return _programs["p"]


def _wrap16(ids: np.ndarray) -> np.ndarray:
    """Index list -> [128, n/16] int16 tile block (idx i at [i%16, i//16],
    replicated 8x down the partitions for the 8 Q7 cores)."""
    n = ids.shape[0]
    w = ids.reshape(n // 16, 16).T.astype(np.int16)  # [16, n/16]
    return np.tile(w, (8, 1))


def _prep(x: np.ndarray, user_table: np.ndarray, item_table: np.ndarray):
    """Sort/deal batch rows, build per-core idx tiles + table windows.

    Returns (in_maps, perm) where perm[k][i] is the batch row computed at
    position i of core k.
    """
    u_ids = x[:, 0].astype(np.int64)
    v_ids = x[:, 1].astype(np.int64)
    order = np.argsort(u_ids, kind="stable")
    in_maps = []
    perm = np.empty((N_CORES, BPC), dtype=np.int64)
    for k in range(N_CORES):
        sel = order[k * BPC : (k + 1) * BPC]
        sub = sel[np.argsort(v_ids[sel], kind="stable")]
        perm[k] = sub
        cu = u_ids[sub]
        cv = v_ids[sub]

        u_base = int(cu.min())
        if int(cu.max()) - u_base >= WIN:
            raise ValueError("user id span exceeds int16 gather window")

        tbl = np.zeros(((1 + VRUNS) * WIN, D), dtype=np.float32)
        take = min(WIN, user_table.shape[0] - u_base)
        tbl[:take] = user_table[u_base : u_base + take]

        idx_blocks = []
        for r in range(VRUNS):
            idx_blocks.append(_wrap16(cu[r * VRUN : (r + 1) * VRUN] - u_base))
            seg = cv[r * VRUN : (r + 1) * VRUN]
            v_base = int(seg[0])  # sorted ascending
            if int(seg[-1]) - v_base >= WIN:
                raise ValueError("item id span exceeds int16 gather window")
            idx_blocks.append(_wrap16(seg - v_base))
            take = min(WIN, item_table.shape[0] - v_base)
            tbl[(1 + r) * WIN : (1 + r) * WIN + take] = item_table[
                v_base : v_base + take
            ]

        in_maps.append(
            {
                "idx": np.ascontiguousarray(np.concatenate(idx_blocks, axis=1)),
                "tbl": tbl,
            }
        )
    return in_maps, perm


def _run(x, user_table, item_table, **run_kwargs):
    x = np.asarray(x)
    ut = np.asarray(user_table, dtype=np.float32)
    it = np.asarray(item_table, dtype=np.float32)
    assert x.shape == (B, 2), x.shape
    in_maps, perm = _prep(x, ut, it)
    nc = _get_program()
    res = run_bass_kernel_spmd(nc, in_maps, list(range(N_CORES)), **run_kwargs)
    out = np.empty(B, np.float32)
    for k in range(N_CORES):
        out[perm[k]] = res.results[k]["out"].T.ravel()
    return out, res


def kernel(x, user_table, item_table):
    out, _ = _run(x, user_table, item_table)
    return out


# revision 25
# speedup vs baseline: 1.0047x; 1.0047x over previous
"""Embedding-lookup MF model kernel for Trainium2 (8 NeuronCores).

reference math (B = 16384, D = 64):
    u   = user_table[x[:, 0]]          # [B, D]
    v   = item_table[x[:, 1]]          # [B, D]
    out = sigmoid(sum(u * v, -1))      # [B]

Strategy: data-parallel across the batch, with HOST-side index prep that
lets each core fetch its 4096 rows with 8 InstDMAGatherAnt instructions
spread over the 4 SWDGE queues. Q7 descriptor generation costs ~9 ns per
256 B row-descriptor and queue r dispatches to its own Q7 core pair, so 4
queues generate concurrently (~1024 descriptors each, ~4.7 us) — vs ~36 us
for the 32 serialized per-partition indirect DMAs of the naive kernel.

dma_gather takes int16 indices (< 32768) into a row window whose base is a
compile-time AP offset, and writes gathered row i of the instruction to
dst[i % 128, i // 128, :]. To make every index fit in 16 bits:

  - batch rows are sorted globally by user id and dealt to the 8 cores in
    2048-row quantile spans: each core's user ids then span ~12.5k values
    (< 32768 with huge margin), so all u-gathers read one per-core user
    window (sliced host-side and uploaded per core).
  - within a core, its 2048 rows are sorted by item id; the 4 runs of 512
    consecutive sorted item ids each span ~25k values (< 32768), giving
    one v-gather per run from a per-run item window.

Queue r handles positions [512r, 512r+512): one u-gather + one v-gather.

Per-core uploaded table: [5 * 32768, 64] f32 = 40 MB (u window + 4 v
windows). Index tile: [128, 256] int16 (idx i of an instruction lives at
partition i%16, col i//16, replicated 8x down the partition dim for the 8
Q7 cores). Host un-permutes the [128, 16] result tiles at the end.

The mlp Q7 library (which provides dma_gather) is loaded up front; its
~6.5 us IRAM reload overlaps the idx load. Each run's DVE mul + reduce,
ACT sigmoid, and output store pipeline behind the gathers, so only the
last run's chain sits on the tail.
"""

import os

# A previously crashed process can leave the NeuronCores wedged
# (NRT_EXEC_UNIT_UNRECOVERABLE on the next run); requesting a core reset at
# runtime init is harmless otherwise and self-heals that state.
os.environ.setdefault("NEURON_RT_RESET_CORES", "1")

import numpy as np

import concourse.mybir as mybir
import concourse.tile as tile
from concourse import bacc, library_config
from concourse.bass_utils import run_bass_kernel_spmd

N_CORES = 8
P = 128
D = 64
B = 16384
BPC = B // N_CORES  # 2048 batch rows per core
NBLK = BPC // P  # 16 column blocks of 128 batch rows
WIN = 32768  # dma_gather int16 index window (rows)
VRUNS = 4
VRUN = BPC // VRUNS  # 512 positions per v-run
VBLK = VRUN // P  # 4 blocks per v-run
UCOLS = BPC // 16  # 128 idx columns for the u gather
VCOLS = VRUN // 16  # 32 idx columns per v run

_programs: dict = {}


def _build():
    """Single-core program, run SPMD on 8 cores."""
    nc = bacc.Bacc(
        "TRN2",
        target_bir_lowering=False,
        debug=False,
        detect_race_conditions=False,
        num_swdge_queues=4,
    )
    idx = nc.dram_tensor(
        "idx", [P, UCOLS + VRUNS * VCOLS], mybir.dt.int16, kind="ExternalInput"
    )
    tbl = nc.dram_tensor(
        "tbl", [(1 + VRUNS) * WIN, D], mybir.dt.float32, kind="ExternalInput"
    )
    out = nc.dram_tensor("out", [P, NBLK], mybir.dt.float32, kind="ExternalOutput")

    with tile.TileContext(nc) as tc:
        with (
            tc.tile_pool(name="io", bufs=1) as io_pool,
            tc.tile_pool(name="prod", bufs=2) as prod_pool,
        ):
            # Q7 IRAM reload for the gather library; blocks the Q7 cluster
            # ~6.5 us, overlapping the idx transfer below
            nc.gpsimd.load_library(library_config.mlp)
            t_idx = io_pool.tile([P, UCOLS + VRUNS * VCOLS], mybir.dt.int16)
            nc.scalar.dma_start(out=t_idx[:], in_=idx[:])
            tu = io_pool.tile([P, BPC // P * D], mybir.dt.float32)
            tv = io_pool.tile([P, BPC // P * D], mybir.dt.float32)
            t_res = io_pool.tile([P, NBLK], mybir.dt.float32)
            t_bias = io_pool.tile([P, 1], mybir.dt.float32)
            nc.vector.memset(t_bias[:], 0.0)

            tu3 = tu[:].rearrange("p (n d) -> p n d", d=D)
            tv3 = tv[:].rearrange("p (n d) -> p n d", d=D)

            # Q7 descriptor generation runs at ~9 ns/descriptor per core
            # pair, and SWDGE queue r dispatches to its own core pair — so
            # split the work into a (u, v) gather pair per queue: 4 pairs
            # generate concurrently, ~1024 descriptors each. Queue 0's
            # pushes block the Pool sequencer until its pair consumes them,
            # so issue queue 0 last — queues 1-3 then start without the
            # ~4.6 us stagger.
            for r in [1, 2, 3, 0]:
                nc.gpsimd.dma_gather(
                    tu3[:, r * VBLK : (r + 1) * VBLK, :],
                    tbl[0:WIN, :],
                    t_idx[:, r * 2 * VCOLS : r * 2 * VCOLS + VCOLS],
                    VRUN,
                    VRUN,
                    D,
                    queue_num=r,
                )
            # v gathers go in two half-waves of 256 rows: the first wave's
            # data lands while the second wave generates, so the DVE mul /
            # reduce + ACT sigmoid + store pipeline overlaps it instead of
            # bunching after the last transfer. A 512-entry wrapped idx
            # block's first 256 entries occupy its first 16 columns, so the
            # halves are column-contiguous slices of the same idx block.
            HBLK = VBLK // 2
            for h in range(2):
                for r in [1, 2, 3, 0]:
                    c0 = r * 2 * VCOLS + VCOLS + h * (VCOLS // 2)
                    b0 = r * VBLK + h * HBLK
                    nc.gpsimd.dma_gather(
                        tv3[:, b0 : b0 + HBLK, :],
                        tbl[(1 + r) * WIN : (2 + r) * WIN, :],
                        t_idx[:, c0 : c0 + VCOLS // 2],
                        VRUN // 2,
                        VRUN // 2,
                        D,
                        queue_num=r,
                    )
                    w = prod_pool.tile([P, HBLK * D], mybir.dt.float32, tag="w")
                    nc.vector.tensor_mul(
                        out=w[:],
                        in0=tu[:, b0 * D : (b0 + HBLK) * D],
                        in1=tv[:, b0 * D : (b0 + HBLK) * D],
                    )
                    rs = t_res[:, b0 : b0 + HBLK]
                    nc.vector.reduce_sum(
                        out=rs,
                        in_=w[:].rearrange("p (n d) -> p n d", d=D),
                        axis=mybir.AxisListType.X,
                    )
                    nc.scalar.activation(
                        out=rs,
                        in_=rs,
                        func=mybir.ActivationFunctionType.Sigmoid,
                        bias=t_bias[:],
                    )
                    nc.sync.dma_start(out=out[:, b0 : b0 + HBLK], in_=rs)

    nc.compile()
    return nc


def _get_program():
    if "p" not in _programs:
        _programs["p"] = _build()
    return _programs["p"]


def _wrap16(ids: np.ndarray) -> np.ndarray:
    """Index list -> [128, n/16] int16 tile block (idx i at [i%16, i//16],
    replicated 8x down the partitions for the 8 Q7 cores)."""
    n = ids.shape[0]
    w = ids.reshape(n // 16, 16).T.astype(np.int16)  # [16, n/16]
    return np.tile(w, (8, 1))


def _prep(x: np.ndarray, user_table: np.ndarray, item_table: np.ndarray):
    """Sort/deal batch rows, build per-core idx tiles + table windows.

    Returns (in_maps, perm) where perm[k][i] is the batch row computed at
    position i of core k.
    """
    u_ids = x[:, 0].astype(np.int64)
    v_ids = x[:, 1].astype(np.int64)
    order = np.argsort(u_ids, kind="stable")
    in_maps = []
    perm = np.empty((N_CORES, BPC), dtype=np.int64)
    for k in range(N_CORES):
        sel = order[k * BPC : (k + 1) * BPC]
        sub = sel[np.argsort(v_ids[sel], kind="stable")]
        perm[k] = sub
        cu = u_ids[sub]
        cv = v_ids[sub]

        u_base = int(cu.min())
        if int(cu.max()) - u_base >= WIN:
            raise ValueError("user id span exceeds int16 gather window")

        tbl = np.zeros(((1 + VRUNS) * WIN, D), dtype=np.float32)
        take = min(WIN, user_table.shape[0] - u_base)
        tbl[:take] = user_table[u_base : u_base + take]

        idx_blocks = []
        for r in range(VRUNS):
            idx_blocks.append(_wrap16(cu[r * VRUN : (r + 1) * VRUN] - u_base))
            seg = cv[r * VRUN : (r + 1) * VRUN]
            v_base = int(seg[0])  # sorted ascending
            if int(seg[-1]) - v_base >= WIN:
                raise ValueError("item id span exceeds int16 gather window")
            idx_blocks.append(_wrap16(seg - v_base))
            take = min(WIN, item_table.shape[0] - v_base)
            tbl[(1 + r) * WIN : (1 + r) * WIN + take] = item_table[
                v_base : v_base + take
            ]

        in_maps.append(
            {
                "idx": np.ascontiguousarray(np.concatenate(idx_blocks, axis=1)),
                "tbl": tbl,
            }
        )
    return in_maps, perm


def _run(x, user_table, item_table, **run_kwargs):
    x = np.asarray(x)
    ut = np.asarray(user_table, dtype=np.float32)
    it = np.asarray(item_table, dtype=np.float32)
    assert x.shape == (B, 2), x.shape
    in_maps, perm = _prep(x, ut, it)
    nc = _get_program()
    res = run_bass_kernel_spmd(nc, in_maps, list(range(N_CORES)), **run_kwargs)
    out = np.empty(B, np.float32)
    for k in range(N_CORES):
        out[perm[k]] = res.results[k]["out"].T.ravel()
    return out, res


def kernel(x, user_table, item_table):
    out, _ = _run(x, user_table, item_table)
    return out
